# revision 5
# baseline (speedup 1.0000x reference)
"""MoE experts kernel for Trainium2 (Bass/Tile), expert-parallel across 8 NeuronCores.

Problem: nn_CompressedMoeExperts — T=2048 tokens, D=1024, FF=1536, E=8 experts,
top-k=2.  out[t] = sum_e combine[e,t] * (silu(h[t] @ Wg[e].T) * (h[t] @ Wu[e].T)) @ Wd[e].T

Sharding: expert-parallel with FF-split load balancing.  Each expert's MLP is
split into two FF-half shards; the 16 shards are sorted by routed-token count
and dealt out so every core gets one "big" (slot 0) and one "small" (slot 1)
shard.  Slot capacities are EXACT token counts (padded only to 8 for DMA row
alignment), not 128-multiples: matmul free dims are arbitrary, and phase 2
keeps the down-proj weights stationary with the activations moving so its
cycle count also scales with the exact count.  vs 128-padded capacities this
cuts PE work ~13% ((640+512) -> (528+472) for the seed-0 routing).

Phase 1 computes act = silu(x@WgT) * (x@WuT) * combine_weight (the combine
weight is token-broadcast and folded in here, since phase 2's D-major output
layout has tokens on the free axis where per-partition scalars can't reach).
Phase 2 computes y[d, t] = sum_ff Wd[d, ff] * act[ff, t] with Wd stationary.

Matmuls run fp16 (fast weight load, 1 cycle/row) accumulating fp32 in PSUM.
fp8 was measured at 3.8-5.4% rel err on this input — over the 2e-2 gate.

Startup: 3 dummy warm matmuls (HAM clock warm-up) bridge the first DMAs, then
real matmuls begin as soon as the first weight block + first 128 token
columns land; slot 0's token feed is split into a small head tile so the
first matmul group needs only 0.5 MB in flight.  Input DMAs are dispatched
from BOTH hardware-DGE engines (sync + scalar) to halve dispatch
serialization (~600ns per DMA on one engine).  The last phase-2 chunk is a
narrow 96-token group so the final eviction+DMA tail is short.
"""

import os
import sys

sys.path.insert(0, "/opt/trn_rl_repo")

import numpy as np

import concourse.bass as bass
import concourse.mybir as mybir
import concourse.tile as tile
from concourse import bacc
from concourse.bass_utils import run_bass_kernel_spmd

# Fixed problem shape
T, D, FF, E, TOPK = 2048, 1024, 1536, 8, 2
P = 128
DSUB = D // P     # 8   k-subtiles over the D contraction
FBLK = FF // P    # 12  128-row blocks over the full FF dimension
NSPLIT = 2        # FF-halves per expert (= shard slots per core)
FBH = FBLK // NSPLIT   # 6 128-row FF blocks per shard
FH = FF // NSPLIT      # 768 FF columns per shard
NDB = D // P      # 8   128-col D blocks (phase-2 stationary tiles)
HEAD = 128        # slot-0 token-feed head tile (startup-critical columns)
TAIL = 96         # final phase-2 chunk width (short drain tail)

F32 = mybir.dt.float32
F16 = mybir.dt.float16

_program_cache: dict[tuple, "bass.Bass"] = {}
last_results = None  # BassKernelResults of the most recent run (for profiling)


def _bal_chunks(C: int, cap: int = 512) -> list[int]:
    """Split C into balanced matmul moving-dim chunks of <= cap (PSUM bank
    limit for fp32 accumulation)."""
    n = -(-C // cap)
    base, rem = divmod(C, n)
    return [base + (1 if i < rem else 0) for i in range(n)]


def _build_program(Cs: tuple) -> "bass.Bass":
    C0, C1 = Cs
    nc = bacc.Bacc(None, target_bir_lowering=False)

    xt0a_d = nc.dram_tensor("xt0a", [P, DSUB, HEAD], F16, kind="ExternalInput")
    xt0b_d = nc.dram_tensor("xt0b", [P, DSUB, C0 - HEAD], F16, kind="ExternalInput")
    xt1_d = nc.dram_tensor("xt1", [P, DSUB, C1], F16, kind="ExternalInput")
    wg_d = nc.dram_tensor("wg", [FBLK, P, DSUB, P], F16, kind="ExternalInput")
    wu_d = nc.dram_tensor("wu", [FBLK, P, DSUB, P], F16, kind="ExternalInput")
    wd_d = [
        nc.dram_tensor(f"wd{s}", [P, FBH, NDB, P], F16, kind="ExternalInput")
        for s in range(NSPLIT)
    ]
    wtb_d = [
        nc.dram_tensor(f"wtb{s}", [P, Cs[s]], F16, kind="ExternalInput")
        for s in range(NSPLIT)
    ]
    y_d = [
        nc.dram_tensor(f"y{s}", [NDB, P, Cs[s]], F32, kind="ExternalOutput")
        for s in range(NSPLIT)
    ]

    # phase-1 chunk plans: (token-tile index, offset in tile, width, act col)
    # tile index: 0 = xt0a, 1 = xt0b, 2 = xt1
    p1_chunks = {0: [(0, 0, HEAD, 0)], 1: []}
    col = HEAD
    for cs in _bal_chunks(C0 - HEAD):
        p1_chunks[0].append((1, col - HEAD, cs, col))
        col += cs
    col = 0
    for cs in _bal_chunks(C1):
        p1_chunks[1].append((2, col, cs, col))
        col += cs

    # phase-2 chunk plans: (col0, width); final chunk of the final slot is
    # narrow so the last eviction + output DMA drain is short.
    def p2_plan(C, final):
        if final and C > TAIL + 32:
            widths = _bal_chunks(C - TAIL) + [TAIL]
        else:
            widths = _bal_chunks(C)
        out, c = [], 0
        for w in widths:
            out.append((c, w))
            c += w
        return out

    p2_chunks = {0: p2_plan(C0, False), 1: p2_plan(C1, True)}

    with tile.TileContext(nc) as tc:
        with (
            tc.tile_pool(name="const", bufs=1) as const_pool,
            tc.tile_pool(name="wpool", bufs=3) as wpool,
            tc.tile_pool(name="actp", bufs=1) as act_pool,
            tc.tile_pool(name="sgp", bufs=3) as sg_pool,
            tc.tile_pool(name="yp", bufs=3) as y_pool,
            tc.tile_pool(name="psum", bufs=2, space="PSUM") as psum_pool,
            tc.tile_pool(name="psum_y", bufs=3, space="PSUM") as psum_y_pool,
            tc.tile_pool(name="psum_w", bufs=1, space="PSUM") as psum_w_pool,
        ):
            # HAM pre-warm: a few dummy matmuls (only dep: the memset) keep
            # the PE busy while the first DMAs stage, so real matmuls start
            # near 2.4GHz.  Short: real work takes over as soon as data lands.
            warm_in = const_pool.tile([P, 512], F16)
            nc.gpsimd.memset(warm_in[:], 0.0)
            warm_ps = psum_w_pool.tile([P, 512], F32)
            for _ in range(3):
                nc.tensor.matmul(warm_ps[:], warm_in[:, :P], warm_in[:])

            # --- startup DMAs ---------------------------------------------
            # sync (HWDGE): slot-0 critical path, in demand order.
            wg_tiles = {}
            wu_tiles = {}
            wg_tiles[0] = wpool.tile([P, DSUB, P], F16, tag="wg", name="wg0")
            nc.sync.dma_start(wg_tiles[0][:], wg_d[0])
            xt = [
                const_pool.tile([P, DSUB, HEAD], F16, name="xt0a"),
                const_pool.tile([P, DSUB, C0 - HEAD], F16, name="xt0b"),
                const_pool.tile([P, DSUB, C1], F16, name="xt1"),
            ]
            nc.sync.dma_start(xt[0][:], xt0a_d[:])
            wu_tiles[0] = wpool.tile([P, DSUB, P], F16, tag="wu", name="wu0")
            nc.sync.dma_start(wu_tiles[0][:], wu_d[0])
            nc.sync.dma_start(xt[1][:], xt0b_d[:])
            wg_tiles[1] = wpool.tile([P, DSUB, P], F16, tag="wg", name="wg1")
            nc.sync.dma_start(wg_tiles[1][:], wg_d[1])
            wu_tiles[1] = wpool.tile([P, DSUB, P], F16, tag="wu", name="wu1")
            nc.sync.dma_start(wu_tiles[1][:], wu_d[1])

            # scalar (the other HWDGE engine): non-critical startup feeds.
            wtb_sb = [
                const_pool.tile([P, Cs[s]], F16, name=f"wtb{s}")
                for s in range(NSPLIT)
            ]
            nc.scalar.dma_start(wtb_sb[0][:], wtb_d[0][:])
            nc.scalar.dma_start(wtb_sb[1][:], wtb_d[1][:])
            nc.scalar.dma_start(xt[2][:], xt1_d[:])

            wd_sb = [
                const_pool.tile([P, FBH, NDB, P], F16, name=f"wd{s}")
                for s in range(NSPLIT)
            ]
            act = [
                act_pool.tile([P, FBH, Cs[s]], F16, name=f"act{s}")
                for s in range(NSPLIT)
            ]

            # deferred scalar-engine DMA schedule, keyed by global fb index:
            # wd piece j = (slot j//FBH, fs j%FBH); wg/wu for fb 6..11.
            wd_sched = {2: [0, 1], 3: [2, 3]}
            for i in range(4, 12):
                wd_sched[i] = [i]

            for s in range(NSPLIT):
                C = Cs[s]
                # Phase 1: gateT/upT per FF-block, fused silu*up*combine_wt
                for fbl in range(FBH):
                    fb = s * FBH + fbl
                    wg_t = wg_tiles.pop(fb)
                    wu_t = wu_tiles.pop(fb)
                    # prefetch fb+2 weights: sync for fb 2..5, scalar later
                    nf = fb + 2
                    if nf < FBLK:
                        eng = nc.sync if nf <= 5 else nc.scalar
                        nwg = wpool.tile([P, DSUB, P], F16, tag="wg", name="wg")
                        eng.dma_start(nwg[:], wg_d[nf])
                        nwu = wpool.tile([P, DSUB, P], F16, tag="wu", name="wu")
                        eng.dma_start(nwu[:], wu_d[nf])
                        wg_tiles[nf] = nwg
                        wu_tiles[nf] = nwu
                    for j in wd_sched.get(fb, []):
                        nc.scalar.dma_start(
                            wd_sb[j // FBH][:, j % FBH], wd_d[j // FBH][:, j % FBH]
                        )

                    for (ti, toff, cs, acol) in p1_chunks[s]:
                        src = xt[ti]
                        pg = psum_pool.tile([P, 512], F32, tag="pg", name="pg")[:, :cs]
                        pu = psum_pool.tile([P, 512], F32, tag="pu", name="pu")[:, :cs]
                        for k in range(DSUB):
                            nc.tensor.matmul(
                                pg,
                                wg_t[:, k, :],
                                src[:, k, toff : toff + cs],
                                start=(k == 0),
                                stop=(k == DSUB - 1),
                            )
                        for k in range(DSUB):
                            nc.tensor.matmul(
                                pu,
                                wu_t[:, k, :],
                                src[:, k, toff : toff + cs],
                                start=(k == 0),
                                stop=(k == DSUB - 1),
                            )
                        sg = sg_pool.tile([P, 512], F32, tag="sg", name="sg")[:, :cs]
                        nc.scalar.activation(
                            sg, pg, mybir.ActivationFunctionType.Silu
                        )
                        tm = sg_pool.tile([P, 512], F32, tag="tm", name="tm")[:, :cs]
                        nc.vector.tensor_mul(tm, pu, wtb_sb[s][:, acol : acol + cs])
                        nc.vector.tensor_mul(
                            act[s][:, fbl, acol : acol + cs], sg, tm
                        )

                # Phase 2: y[d, t] = sum_ff Wd_half[d, ff] * act[ff, t]
                for dblk in range(NDB):
                    for (c0, cs) in p2_chunks[s]:
                        py = psum_y_pool.tile([P, 512], F32, tag="py", name="py")[:, :cs]
                        for fs in range(FBH):
                            nc.tensor.matmul(
                                py,
                                wd_sb[s][:, fs, dblk, :],
                                act[s][:, fs, c0 : c0 + cs],
                                start=(fs == 0),
                                stop=(fs == FBH - 1),
                            )
                        y_sb = y_pool.tile([P, 512], F32, tag="ysb", name="ysb")[:, :cs]
                        nc.vector.tensor_copy(y_sb, py)
                        nc.sync.dma_start(y_d[s][dblk, :, c0 : c0 + cs], y_sb)

    nc.compile()
    return nc


def _shard_feed(h16, gp, up, dp, combine, routed, e, piece, C):
    """Build one (expert, FF-half) shard's DMA feeds, pre-laid-out to match the
    kernel's SBUF tile layouts exactly (every DMA contiguous)."""
    r = routed[e]
    n_e = len(r)
    idx_pad = np.zeros(C, np.int64)
    idx_pad[:n_e] = r
    wt_pad = np.zeros(C, np.float32)
    wt_pad[:n_e] = combine[e, r]
    hs = slice(piece * FH, (piece + 1) * FH)

    xg = h16[idx_pad]  # [C, D] fp16
    xt_feed = np.ascontiguousarray(xg.reshape(C, DSUB, P).transpose(2, 1, 0))
    wg_feed = np.ascontiguousarray(
        gp[e][hs, :].astype(np.float16).reshape(FBH, P, DSUB, P).transpose(0, 3, 2, 1)
    )
    wu_feed = np.ascontiguousarray(
        up[e][hs, :].astype(np.float16).reshape(FBH, P, DSUB, P).transpose(0, 3, 2, 1)
    )
    # wd_feed[p, fs, dblk, j] = down_proj[e][dblk*P+j, half*FH + fs*P + p]
    wd_feed = np.ascontiguousarray(
        dp[e][:, hs].astype(np.float16).reshape(NDB, P, FBH, P).transpose(3, 2, 0, 1)
    )
    wtb_feed = np.ascontiguousarray(
        np.broadcast_to(wt_pad.astype(np.float16), (P, C))
    )
    return xt_feed, wg_feed, wu_feed, wd_feed, wtb_feed


def kernel(hidden_states, top_k_index, top_k_weights, gate_proj, up_proj, down_proj):
    h = np.ascontiguousarray(np.asarray(hidden_states, dtype=np.float32))
    idx = np.asarray(top_k_index)
    wts = np.asarray(top_k_weights, dtype=np.float32)
    gp = np.asarray(gate_proj, dtype=np.float32)
    up = np.asarray(up_proj, dtype=np.float32)
    dp = np.asarray(down_proj, dtype=np.float32)
    assert h.shape == (T, D) and idx.shape == (T, TOPK)
    assert gp.shape == (E, FF, D) and dp.shape == (E, D, FF)

    # combine[e, t] = sum_k wts[t, k] * (idx[t, k] == e)
    combine = np.zeros((E, T), np.float32)
    for k in range(TOPK):
        np.add.at(combine, (idx[:, k], np.arange(T)), wts[:, k])

    routed = [np.nonzero(combine[e] > 0)[0] for e in range(E)]
    cnt = [len(r) for r in routed]

    # E*NSPLIT (expert, FF-half) shards, sorted by routed count; slot s takes
    # ranks [s*E, (s+1)*E).  Capacities are the exact per-tier max counts,
    # aligned to 8 tokens for 16B DMA rows (must also keep HEAD columns in
    # the slot-0 head tile and a nonzero tail chunk).
    shards = sorted(
        ((e, piece) for e in range(E) for piece in range(NSPLIT)),
        key=lambda sh: -cnt[sh[0]],
    )
    slots = [shards[s * E : (s + 1) * E] for s in range(NSPLIT)]
    pad8 = lambda n: -(-n // 8) * 8
    Cs = (
        max(HEAD + 8, pad8(max(cnt[e] for e, _ in slots[0]))),
        max(TAIL + 40, pad8(max(cnt[e] for e, _ in slots[1]))),
    )

    h16 = h.astype(np.float16)
    in_maps = []
    for core in range(E):
        m = {}
        for s in range(NSPLIT):
            e, piece = slots[s][core]
            xt_f, wg_f, wu_f, wd_f, wtb_f = _shard_feed(
                h16, gp, up, dp, combine, routed, e, piece, Cs[s]
            )
            if s == 0:
                m["xt0a"] = np.ascontiguousarray(xt_f[:, :, :HEAD])
                m["xt0b"] = np.ascontiguousarray(xt_f[:, :, HEAD:])
                wg_parts, wu_parts = [wg_f], [wu_f]
            else:
                m["xt1"] = xt_f
                wg_parts.append(wg_f)
                wu_parts.append(wu_f)
            m[f"wd{s}"] = wd_f
            m[f"wtb{s}"] = wtb_f
        m["wg"] = np.ascontiguousarray(np.concatenate(wg_parts, axis=0))
        m["wu"] = np.ascontiguousarray(np.concatenate(wu_parts, axis=0))
        in_maps.append(m)

    ys = _run_on_device(Cs, in_maps)

    out = np.zeros((T, D), np.float32)
    for core in range(E):
        for s in range(NSPLIT):
            e, piece = slots[s][core]
            r = routed[e]
            # y[s] is [NDB, P, C] fp32, D-major: D = dblk*P + p
            yv = ys[core][s].reshape(D, Cs[s])
            out[r] += yv[:, : len(r)].T
    return out


def _have_axon() -> bool:
    """The bass kernel executes via PJRT on the axon-tunneled NeuronCores.
    If the calling process pinned JAX_PLATFORMS=cpu (hiding them), fall back
    to a clean subprocess."""
    try:
        import jax

        return sum(1 for d in jax.devices() if getattr(d, "platform", "") != "cpu") >= E
    except Exception:
        return False


def _run_on_device(Cs: tuple, in_maps: list) -> list:
    global last_results
    if _have_axon():
        if Cs not in _program_cache:
            _program_cache[Cs] = _build_program(Cs)
        nc = _program_cache[Cs]
        last_results = run_bass_kernel_spmd(nc, in_maps, core_ids=list(range(E)))
        return [
            [last_results.results[c][f"y{s}"] for s in range(NSPLIT)]
            for c in range(E)
        ]

    import pickle
    import subprocess
    import tempfile

    d = tempfile.mkdtemp()
    inp, outp = os.path.join(d, "in.pkl"), os.path.join(d, "out.pkl")
    with open(inp, "wb") as f:
        pickle.dump((Cs, in_maps), f)
    env = dict(os.environ)
    env.pop("JAX_PLATFORMS", None)
    subprocess.run(
        [sys.executable, os.path.abspath(__file__), "--device-run", inp, outp],
        check=True,
        env=env,
    )
    with open(outp, "rb") as f:
        return pickle.load(f)


if __name__ == "__main__" and "--device-run" in sys.argv:
    import pickle

    _inp, _outp = sys.argv[2], sys.argv[3]
    with open(_inp, "rb") as f:
        _Cs, _in_maps = pickle.load(f)
    _nc = _build_program(_Cs)
    _res = run_bass_kernel_spmd(_nc, _in_maps, core_ids=list(range(E)))
    with open(_outp, "wb") as f:
        pickle.dump(
            [[_res.results[c][f"y{s}"] for s in range(NSPLIT)] for c in range(E)],
            f,
        )


# revision 16
# speedup vs baseline: 1.0499x; 1.0499x over previous
"""MoE experts kernel for Trainium2 (Bass/Tile), expert-parallel across 8 NeuronCores.

Problem: nn_CompressedMoeExperts — T=2048 tokens, D=1024, FF=1536, E=8 experts,
top-k=2.  out[t] = sum_e combine[e,t] * (silu(h[t] @ Wg[e].T) * (h[t] @ Wu[e].T)) @ Wd[e].T

Sharding: expert-parallel with FF-split load balancing.  Each expert's MLP is
split into two FF-half shards; the 16 shards are sorted by routed-token count
and dealt out so every core gets one "big" (slot 0) and one "small" (slot 1)
shard.  Slot capacities are EXACT token counts (padded only to 8 for DMA row
alignment), not 128-multiples: matmul free dims are arbitrary, and phase 2
keeps the down-proj weights stationary with the activations moving so its
cycle count also scales with the exact count.  vs 128-padded capacities this
cuts PE work ~13% ((640+512) -> (528+472) for the seed-0 routing).

Phase 1 computes act = silu(x@WgT) * (x@WuT) * combine_weight (the combine
weight is token-broadcast and folded in here, since phase 2's D-major output
layout has tokens on the free axis where per-partition scalars can't reach).
Phase 2 computes y[d, t] = sum_ff Wd[d, ff] * act[ff, t] with Wd stationary.

Matmuls run fp16 (fast weight load, 1 cycle/row) accumulating fp32 in PSUM.
fp8 was measured at 3.8-5.4% rel err on this input — over the 2e-2 gate.

Startup: 3 dummy warm matmuls (HAM clock warm-up) bridge the first DMAs, then
real matmuls begin as soon as the first weight block + first 128 token
columns land; slot 0's token feed is split into a small head tile so the
first matmul group needs only 0.5 MB in flight.  Input DMAs are dispatched
from BOTH hardware-DGE engines (sync + scalar) to halve dispatch
serialization (~600ns per DMA on one engine).  The last phase-2 chunk is a
narrow 96-token group so the final eviction+DMA tail is short.
"""

import os
import sys

sys.path.insert(0, "/opt/trn_rl_repo")

import numpy as np

import concourse.bass as bass
import concourse.mybir as mybir
import concourse.tile as tile
from concourse import bacc
from concourse.bass_utils import run_bass_kernel_spmd

# Fixed problem shape
T, D, FF, E, TOPK = 2048, 1024, 1536, 8, 2
P = 128
DSUB = D // P     # 8   k-subtiles over the D contraction
FBLK = FF // P    # 12  128-row blocks over the full FF dimension
NSPLIT = 2        # FF-halves per expert (= shard slots per core)
FBH = FBLK // NSPLIT   # 6 128-row FF blocks per shard
FH = FF // NSPLIT      # 768 FF columns per shard
NDB = D // P      # 8   128-col D blocks (phase-2 stationary tiles)
HEAD = 128        # slot-0 token-feed head tile (startup-critical columns)
TAIL = 96         # final phase-2 chunk width (short drain tail)

F32 = mybir.dt.float32
F16 = mybir.dt.float16

_program_cache: dict[tuple, "bass.Bass"] = {}
last_results = None  # BassKernelResults of the most recent run (for profiling)


def _bal_chunks(C: int, cap: int = 512) -> list[int]:
    """Split C into balanced matmul moving-dim chunks of <= cap (PSUM bank
    limit for fp32 accumulation)."""
    n = -(-C // cap)
    base, rem = divmod(C, n)
    return [base + (1 if i < rem else 0) for i in range(n)]


def _build_program(Cs: tuple) -> "bass.Bass":
    C0, C1 = Cs
    nc = bacc.Bacc(None, target_bir_lowering=False)

    xt0_d = nc.dram_tensor("xt0", [P, DSUB, C0], F16, kind="ExternalInput")
    xt1_d = nc.dram_tensor("xt1", [P, DSUB, C1], F16, kind="ExternalInput")
    wg_d = nc.dram_tensor("wg", [FBLK, P, DSUB, P], F16, kind="ExternalInput")
    wu_d = nc.dram_tensor("wu", [FBLK, P, DSUB, P], F16, kind="ExternalInput")
    wd_d = [
        nc.dram_tensor(f"wd{s}", [P, FBH, NDB, P], F16, kind="ExternalInput")
        for s in range(NSPLIT)
    ]
    wtb_d = [
        nc.dram_tensor(f"wtb{s}", [P, Cs[s]], F16, kind="ExternalInput")
        for s in range(NSPLIT)
    ]
    y_d = [
        nc.dram_tensor(f"y{s}", [NDB, P, Cs[s]], F32, kind="ExternalOutput")
        for s in range(NSPLIT)
    ]

    # phase-1 chunk plans per slot: (col0, width)
    p1_chunks = {}
    for s, C in enumerate(Cs):
        out, col = [], 0
        for cs in _bal_chunks(C):
            out.append((col, cs))
            col += cs
        p1_chunks[s] = out

    # phase-2 chunk plans: (col0, width); final chunk of the final slot is
    # narrow so the last eviction + output DMA drain is short.
    def p2_plan(C, final):
        if final and C > TAIL + 32:
            widths = _bal_chunks(C - TAIL) + [TAIL]
        else:
            widths = _bal_chunks(C)
        out, c = [], 0
        for w in widths:
            out.append((c, w))
            c += w
        return out

    p2_chunks = {0: p2_plan(C0, False), 1: p2_plan(C1, True)}

    with tile.TileContext(nc) as tc:
        with (
            tc.tile_pool(name="const", bufs=1) as const_pool,
            tc.tile_pool(name="wpool", bufs=3) as wpool,
            tc.tile_pool(name="actp", bufs=1) as act_pool,
            tc.tile_pool(name="sgp", bufs=3) as sg_pool,
            tc.tile_pool(name="yp", bufs=6) as y_pool,
            tc.tile_pool(name="psum", bufs=2, space="PSUM") as psum_pool,
            tc.tile_pool(name="psum_y", bufs=3, space="PSUM") as psum_y_pool,
            tc.tile_pool(name="psum_w", bufs=1, space="PSUM") as psum_w_pool,
        ):
            # HAM pre-warm: dummy matmuls (only dep: the memset) keep the PE
            # busy while the first DMAs stage, so real matmuls start at
            # 2.4GHz.  Sized to the ~1.6MB startup staging time (~5us).
            warm_in = const_pool.tile([P, 512], F16)
            nc.gpsimd.memset(warm_in[:], 0.0)
            warm_ps = psum_w_pool.tile([P, 512], F32)
            for _ in range(10):
                nc.tensor.matmul(warm_ps[:], warm_in[:, :P], warm_in[:])

            # --- startup DMAs ---------------------------------------------
            # sync (HWDGE): slot-0 critical path, in demand order.  xt0 in
            # 4 pieces of 2 k-planes (2KB+ rows, spread across DMA engines).
            wg_tiles = {}
            wu_tiles = {}
            wg_tiles[0] = wpool.tile([P, DSUB, P], F16, tag="wg", name="wg0")
            nc.sync.dma_start(wg_tiles[0][:], wg_d[0])
            xt = [
                const_pool.tile([P, DSUB, C0], F16, name="xt0"),
                const_pool.tile([P, DSUB, C1], F16, name="xt1"),
            ]
            for k in range(DSUB):
                nc.sync.dma_start(xt[0][:, k], xt0_d[:, k])
            wu_tiles[0] = wpool.tile([P, DSUB, P], F16, tag="wu", name="wu0")
            nc.sync.dma_start(wu_tiles[0][:], wu_d[0])
            wg_tiles[1] = wpool.tile([P, DSUB, P], F16, tag="wg", name="wg1")
            nc.sync.dma_start(wg_tiles[1][:], wg_d[1])
            wu_tiles[1] = wpool.tile([P, DSUB, P], F16, tag="wu", name="wu1")
            nc.sync.dma_start(wu_tiles[1][:], wu_d[1])

            # scalar (the other HWDGE engine): only the small combine-weight
            # rows upfront; the 1MB slot-1 token feed and the down-proj
            # weights are deferred into the fb loop to keep startup bandwidth
            # for the slot-0 critical path.
            wtb_sb = [
                const_pool.tile([P, Cs[s]], F16, name=f"wtb{s}")
                for s in range(NSPLIT)
            ]
            nc.scalar.dma_start(wtb_sb[0][:], wtb_d[0][:])
            nc.scalar.dma_start(wtb_sb[1][:], wtb_d[1][:])

            wd_sb = [
                const_pool.tile([P, FBH, NDB, P], F16, name=f"wd{s}")
                for s in range(NSPLIT)
            ]
            act = [
                act_pool.tile([P, FBH, Cs[s]], F16, name=f"act{s}")
                for s in range(NSPLIT)
            ]

            # deferred scalar-engine DMA schedule, keyed by global fb index:
            # wd piece j = (slot j//FBH, fs j%FBH).  Slot-0 pieces (0..5)
            # must all be issued during slot-0 phase 1 (its phase 2 reads
            # them), slot-1 pieces during slot-1 phase 1.
            wd_sched = {1: [0, 1]}
            for i in range(2, 6):
                wd_sched[i] = [i]
            wd_sched[6] = [6, 7]
            for i in range(7, 11):
                wd_sched[i] = [i + 1]

            for s in range(NSPLIT):
                C = Cs[s]
                # Phase 1: gateT/upT per FF-block, fused silu*up*combine_wt
                for fbl in range(FBH):
                    fb = s * FBH + fbl
                    wg_t = wg_tiles.pop(fb)
                    wu_t = wu_tiles.pop(fb)
                    if fb == 0:
                        # slot-1 token feed, now that slot-0 staging is done
                        nc.scalar.dma_start(xt[1][:], xt1_d[:])
                    # prefetch fb+2 weights: sync for fb 2..5, scalar later
                    nf = fb + 2
                    if nf < FBLK:
                        eng = nc.sync if nf <= 5 else nc.scalar
                        nwg = wpool.tile([P, DSUB, P], F16, tag="wg", name="wg")
                        eng.dma_start(nwg[:], wg_d[nf])
                        nwu = wpool.tile([P, DSUB, P], F16, tag="wu", name="wu")
                        eng.dma_start(nwu[:], wu_d[nf])
                        wg_tiles[nf] = nwg
                        wu_tiles[nf] = nwu
                    for j in wd_sched.get(fb, []):
                        nc.scalar.dma_start(
                            wd_sb[j // FBH][:, j % FBH], wd_d[j // FBH][:, j % FBH]
                        )

                    for (toff, cs) in p1_chunks[s]:
                        src = xt[s]
                        pg = psum_pool.tile([P, 512], F32, tag="pg", name="pg")[:, :cs]
                        pu = psum_pool.tile([P, 512], F32, tag="pu", name="pu")[:, :cs]
                        for k in range(DSUB):
                            nc.tensor.matmul(
                                pg,
                                wg_t[:, k, :],
                                src[:, k, toff : toff + cs],
                                start=(k == 0),
                                stop=(k == DSUB - 1),
                            )
                        for k in range(DSUB):
                            nc.tensor.matmul(
                                pu,
                                wu_t[:, k, :],
                                src[:, k, toff : toff + cs],
                                start=(k == 0),
                                stop=(k == DSUB - 1),
                            )
                        sg = sg_pool.tile([P, 512], F32, tag="sg", name="sg")[:, :cs]
                        nc.scalar.activation(
                            sg, pg, mybir.ActivationFunctionType.Silu
                        )
                        tm = sg_pool.tile([P, 512], F32, tag="tm", name="tm")[:, :cs]
                        nc.vector.tensor_mul(tm, pu, wtb_sb[s][:, toff : toff + cs])
                        nc.vector.tensor_mul(
                            act[s][:, fbl, toff : toff + cs], sg, tm
                        )

                # Phase 2: y[d, t] = sum_ff Wd_half[d, ff] * act[ff, t];
                # output DMAs alternate between the two HWDGE engines so the
                # ~600ns dispatch serialization doesn't back up evictions.
                grp = 0
                for dblk in range(NDB):
                    for (c0, cs) in p2_chunks[s]:
                        py = psum_y_pool.tile([P, 512], F32, tag="py", name="py")[:, :cs]
                        for fs in range(FBH):
                            nc.tensor.matmul(
                                py,
                                wd_sb[s][:, fs, dblk, :],
                                act[s][:, fs, c0 : c0 + cs],
                                start=(fs == 0),
                                stop=(fs == FBH - 1),
                            )
                        y_sb = y_pool.tile([P, 512], F32, tag="ysb", name="ysb")[:, :cs]
                        nc.vector.tensor_copy(y_sb, py)
                        eng = nc.sync if grp % 2 == 0 else nc.scalar
                        eng.dma_start(y_d[s][dblk, :, c0 : c0 + cs], y_sb)
                        grp += 1

    nc.compile()
    return nc


def _shard_feed(h16, gp, up, dp, combine, routed, e, piece, C):
    """Build one (expert, FF-half) shard's DMA feeds, pre-laid-out to match the
    kernel's SBUF tile layouts exactly (every DMA contiguous)."""
    r = routed[e]
    n_e = len(r)
    idx_pad = np.zeros(C, np.int64)
    idx_pad[:n_e] = r
    wt_pad = np.zeros(C, np.float32)
    wt_pad[:n_e] = combine[e, r]
    hs = slice(piece * FH, (piece + 1) * FH)

    xg = h16[idx_pad]  # [C, D] fp16
    xt_feed = np.ascontiguousarray(xg.reshape(C, DSUB, P).transpose(2, 1, 0))
    wg_feed = np.ascontiguousarray(
        gp[e][hs, :].astype(np.float16).reshape(FBH, P, DSUB, P).transpose(0, 3, 2, 1)
    )
    wu_feed = np.ascontiguousarray(
        up[e][hs, :].astype(np.float16).reshape(FBH, P, DSUB, P).transpose(0, 3, 2, 1)
    )
    # wd_feed[p, fs, dblk, j] = down_proj[e][dblk*P+j, half*FH + fs*P + p]
    wd_feed = np.ascontiguousarray(
        dp[e][:, hs].astype(np.float16).reshape(NDB, P, FBH, P).transpose(3, 2, 0, 1)
    )
    wtb_feed = np.ascontiguousarray(
        np.broadcast_to(wt_pad.astype(np.float16), (P, C))
    )
    return xt_feed, wg_feed, wu_feed, wd_feed, wtb_feed


def kernel(hidden_states, top_k_index, top_k_weights, gate_proj, up_proj, down_proj):
    h = np.ascontiguousarray(np.asarray(hidden_states, dtype=np.float32))
    idx = np.asarray(top_k_index)
    wts = np.asarray(top_k_weights, dtype=np.float32)
    gp = np.asarray(gate_proj, dtype=np.float32)
    up = np.asarray(up_proj, dtype=np.float32)
    dp = np.asarray(down_proj, dtype=np.float32)
    assert h.shape == (T, D) and idx.shape == (T, TOPK)
    assert gp.shape == (E, FF, D) and dp.shape == (E, D, FF)

    # combine[e, t] = sum_k wts[t, k] * (idx[t, k] == e)
    combine = np.zeros((E, T), np.float32)
    for k in range(TOPK):
        np.add.at(combine, (idx[:, k], np.arange(T)), wts[:, k])

    routed = [np.nonzero(combine[e] > 0)[0] for e in range(E)]
    cnt = [len(r) for r in routed]

    # E*NSPLIT (expert, FF-half) shards, sorted by routed count; slot s takes
    # ranks [s*E, (s+1)*E).  Capacities are the exact per-tier max counts,
    # aligned to 8 tokens for 16B DMA rows (must also keep HEAD columns in
    # the slot-0 head tile and a nonzero tail chunk).
    shards = sorted(
        ((e, piece) for e in range(E) for piece in range(NSPLIT)),
        key=lambda sh: -cnt[sh[0]],
    )
    slots = [shards[s * E : (s + 1) * E] for s in range(NSPLIT)]
    pad8 = lambda n: -(-n // 8) * 8
    Cs = (
        max(136, pad8(max(cnt[e] for e, _ in slots[0]))),
        max(TAIL + 40, pad8(max(cnt[e] for e, _ in slots[1]))),
    )

    h16 = h.astype(np.float16)
    in_maps = []
    for core in range(E):
        m = {}
        for s in range(NSPLIT):
            e, piece = slots[s][core]
            xt_f, wg_f, wu_f, wd_f, wtb_f = _shard_feed(
                h16, gp, up, dp, combine, routed, e, piece, Cs[s]
            )
            if s == 0:
                m["xt0"] = xt_f
                wg_parts, wu_parts = [wg_f], [wu_f]
            else:
                m["xt1"] = xt_f
                wg_parts.append(wg_f)
                wu_parts.append(wu_f)
            m[f"wd{s}"] = wd_f
            m[f"wtb{s}"] = wtb_f
        m["wg"] = np.ascontiguousarray(np.concatenate(wg_parts, axis=0))
        m["wu"] = np.ascontiguousarray(np.concatenate(wu_parts, axis=0))
        in_maps.append(m)

    ys = _run_on_device(Cs, in_maps)

    out = np.zeros((T, D), np.float32)
    for core in range(E):
        for s in range(NSPLIT):
            e, piece = slots[s][core]
            r = routed[e]
            # y[s] is [NDB, P, C] fp32, D-major: D = dblk*P + p
            yv = ys[core][s].reshape(D, Cs[s])
            out[r] += yv[:, : len(r)].T
    return out


def _have_axon() -> bool:
    """The bass kernel executes via PJRT on the axon-tunneled NeuronCores.
    If the calling process pinned JAX_PLATFORMS=cpu (hiding them), fall back
    to a clean subprocess."""
    try:
        import jax

        return sum(1 for d in jax.devices() if getattr(d, "platform", "") != "cpu") >= E
    except Exception:
        return False


def _run_on_device(Cs: tuple, in_maps: list) -> list:
    global last_results
    if _have_axon():
        if Cs not in _program_cache:
            _program_cache[Cs] = _build_program(Cs)
        nc = _program_cache[Cs]
        last_results = run_bass_kernel_spmd(nc, in_maps, core_ids=list(range(E)))
        return [
            [last_results.results[c][f"y{s}"] for s in range(NSPLIT)]
            for c in range(E)
        ]

    import pickle
    import subprocess
    import tempfile

    d = tempfile.mkdtemp()
    inp, outp = os.path.join(d, "in.pkl"), os.path.join(d, "out.pkl")
    with open(inp, "wb") as f:
        pickle.dump((Cs, in_maps), f)
    env = dict(os.environ)
    env.pop("JAX_PLATFORMS", None)
    subprocess.run(
        [sys.executable, os.path.abspath(__file__), "--device-run", inp, outp],
        check=True,
        env=env,
    )
    with open(outp, "rb") as f:
        return pickle.load(f)


if __name__ == "__main__" and "--device-run" in sys.argv:
    import pickle

    _inp, _outp = sys.argv[2], sys.argv[3]
    with open(_inp, "rb") as f:
        _Cs, _in_maps = pickle.load(f)
    _nc = _build_program(_Cs)
    _res = run_bass_kernel_spmd(_nc, _in_maps, core_ids=list(range(E)))
    with open(_outp, "wb") as f:
        pickle.dump(
            [[_res.results[c][f"y{s}"] for s in range(NSPLIT)] for c in range(E)],
            f,
        )


# revision 20
# speedup vs baseline: 1.1193x; 1.0661x over previous
"""MoE experts kernel for Trainium2 (Bass/Tile), expert-parallel across 8 NeuronCores.

Problem: nn_CompressedMoeExperts — T=2048 tokens, D=1024, FF=1536, E=8 experts,
top-k=2.  out[t] = sum_e combine[e,t] * (silu(h[t] @ Wg[e].T) * (h[t] @ Wu[e].T)) @ Wd[e].T

Sharding: expert-parallel with FF-split load balancing.  Each expert's MLP is
split into two FF-half shards; the 16 shards are sorted by routed-token count
and dealt out so every core gets one "big" (slot 0) and one "small" (slot 1)
shard.  Slot capacities are EXACT token counts (padded only to 8 for DMA row
alignment), not 128-multiples: matmul free dims are arbitrary, and phase 2
keeps the down-proj weights stationary with the activations moving so its
cycle count also scales with the exact count.  vs 128-padded capacities this
cuts PE work ~13% ((640+512) -> (528+472) for the seed-0 routing).

Phase 1 computes act = silu(x@WgT) * (x@WuT) * combine_weight (the combine
weight is token-broadcast and folded in here, since phase 2's D-major output
layout has tokens on the free axis where per-partition scalars can't reach).
Phase 2 computes y[d, t] = sum_ff Wd[d, ff] * act[ff, t] with Wd stationary.

Matmuls run fp16 (fast weight load, 1 cycle/row) accumulating fp32 in PSUM.
fp8 was measured at 3.8-5.4% rel err on this input — over the 2e-2 gate.

Startup: 3 dummy warm matmuls (HAM clock warm-up) bridge the first DMAs, then
real matmuls begin as soon as the first weight block + first 128 token
columns land; slot 0's token feed is split into a small head tile so the
first matmul group needs only 0.5 MB in flight.  Input DMAs are dispatched
from BOTH hardware-DGE engines (sync + scalar) to halve dispatch
serialization (~600ns per DMA on one engine).  The last phase-2 chunk is a
narrow 96-token group so the final eviction+DMA tail is short.
"""

import os
import sys

sys.path.insert(0, "/opt/trn_rl_repo")

import numpy as np

import concourse.bass as bass
import concourse.mybir as mybir
import concourse.tile as tile
from concourse import bacc
from concourse.bass_utils import run_bass_kernel_spmd

# Fixed problem shape
T, D, FF, E, TOPK = 2048, 1024, 1536, 8, 2
P = 128
DSUB = D // P     # 8   k-subtiles over the D contraction
FBLK = FF // P    # 12  128-row blocks over the full FF dimension
NSPLIT = 2        # FF-halves per expert (= shard slots per core)
FBH = FBLK // NSPLIT   # 6 128-row FF blocks per shard
FH = FF // NSPLIT      # 768 FF columns per shard
NDB = D // P      # 8   128-col D blocks (phase-2 stationary tiles)
HEAD = 128        # slot-0 token-feed head tile (startup-critical columns)
TAIL = 96         # final phase-2 chunk width (short drain tail)

F32 = mybir.dt.float32
F16 = mybir.dt.float16

_program_cache: dict[tuple, "bass.Bass"] = {}
last_results = None  # BassKernelResults of the most recent run (for profiling)


def _bal_chunks(C: int, cap: int = 512) -> list[int]:
    """Split C into balanced matmul moving-dim chunks of <= cap (PSUM bank
    limit for fp32 accumulation)."""
    n = -(-C // cap)
    base, rem = divmod(C, n)
    return [base + (1 if i < rem else 0) for i in range(n)]


def _build_program(Cs: tuple) -> "bass.Bass":
    C0, C1 = Cs
    nc = bacc.Bacc(None, target_bir_lowering=False)

    xt0_d = nc.dram_tensor("xt0", [P, DSUB, C0], F16, kind="ExternalInput")
    xt1_d = nc.dram_tensor("xt1", [P, DSUB, C1], F16, kind="ExternalInput")
    wg_d = nc.dram_tensor("wg", [FBLK, P, DSUB, P], F16, kind="ExternalInput")
    wu_d = nc.dram_tensor("wu", [FBLK, P, DSUB, P], F16, kind="ExternalInput")
    wd_d = [
        nc.dram_tensor(f"wd{s}", [P, FBH, NDB, P], F16, kind="ExternalInput")
        for s in range(NSPLIT)
    ]
    wtb_d = [
        nc.dram_tensor(f"wtb{s}", [P, Cs[s]], F16, kind="ExternalInput")
        for s in range(NSPLIT)
    ]
    y_d = [
        nc.dram_tensor(f"y{s}", [NDB, P, Cs[s]], F32, kind="ExternalOutput")
        for s in range(NSPLIT)
    ]

    # phase-1 chunk plans per slot: (col0, width)
    p1_chunks = {}
    for s, C in enumerate(Cs):
        out, col = [], 0
        for cs in _bal_chunks(C):
            out.append((col, cs))
            col += cs
        p1_chunks[s] = out

    # phase-2 chunk plans: (col0, width); final chunk of the final slot is
    # narrow so the last eviction + output DMA drain is short.
    def p2_plan(C, final):
        if final and C > TAIL + 32:
            widths = _bal_chunks(C - TAIL) + [TAIL]
        else:
            widths = _bal_chunks(C)
        out, c = [], 0
        for w in widths:
            out.append((c, w))
            c += w
        return out

    p2_chunks = {0: p2_plan(C0, False), 1: p2_plan(C1, True)}

    with tile.TileContext(nc) as tc:
        with (
            tc.tile_pool(name="const", bufs=1) as const_pool,
            tc.tile_pool(name="wpool", bufs=4) as wpool,
            tc.tile_pool(name="actp", bufs=1) as act_pool,
            tc.tile_pool(name="sgp", bufs=3) as sg_pool,
            tc.tile_pool(name="yp", bufs=6) as y_pool,
            tc.tile_pool(name="psum", bufs=2, space="PSUM") as psum_pool,
            tc.tile_pool(name="psum_y", bufs=3, space="PSUM") as psum_y_pool,
            tc.tile_pool(name="psum_w", bufs=1, space="PSUM") as psum_w_pool,
        ):
            # HAM pre-warm: dummy matmuls (only dep: the memset) keep the PE
            # busy while the first DMAs stage, so real matmuls start at
            # 2.4GHz.  Sized to the ~1.6MB startup staging time (~5us).
            warm_in = const_pool.tile([P, 512], F16)
            nc.gpsimd.memset(warm_in[:], 0.0)
            warm_ps = psum_w_pool.tile([P, 512], F32)
            for _ in range(10):
                nc.tensor.matmul(warm_ps[:], warm_in[:, :P], warm_in[:])

            # --- startup DMAs ---------------------------------------------
            # Startup staging is DMA-bandwidth-floored (~1.6MB before fb0
            # can finish), so the critical pieces alternate between BOTH
            # HWDGE engines (sync + scalar) for queue-level parallelism.
            wg_tiles = {}
            wu_tiles = {}
            wtb_sb = [
                const_pool.tile([P, Cs[s]], F16, name=f"wtb{s}")
                for s in range(NSPLIT)
            ]
            wg_tiles[0] = wpool.tile([P, DSUB, P], F16, tag="wg", name="wg0")
            nc.sync.dma_start(wg_tiles[0][:], wg_d[0])
            nc.scalar.dma_start(wtb_sb[0][:], wtb_d[0][:])
            xt = [
                const_pool.tile([P, DSUB, C0], F16, name="xt0"),
                const_pool.tile([P, DSUB, C1], F16, name="xt1"),
            ]
            for k in range(DSUB):
                eng = nc.sync if k % 2 == 0 else nc.scalar
                eng.dma_start(xt[0][:, k], xt0_d[:, k])
            wu_tiles[0] = wpool.tile([P, DSUB, P], F16, tag="wu", name="wu0")
            nc.sync.dma_start(wu_tiles[0][:], wu_d[0])
            wg_tiles[1] = wpool.tile([P, DSUB, P], F16, tag="wg", name="wg1")
            nc.scalar.dma_start(wg_tiles[1][:], wg_d[1])
            wu_tiles[1] = wpool.tile([P, DSUB, P], F16, tag="wu", name="wu1")
            nc.scalar.dma_start(wu_tiles[1][:], wu_d[1])
            nc.scalar.dma_start(wtb_sb[1][:], wtb_d[1][:])

            wd_sb = [
                const_pool.tile([P, FBH, NDB, P], F16, name=f"wd{s}")
                for s in range(NSPLIT)
            ]
            act = [
                act_pool.tile([P, FBH, Cs[s]], F16, name=f"act{s}")
                for s in range(NSPLIT)
            ]

            # deferred scalar-engine DMA schedule, keyed by global fb index:
            # wd piece j = (slot j//FBH, fs j%FBH).  Slot-0 pieces (0..5)
            # must all be issued during slot-0 phase 1 (its phase 2 reads
            # them), slot-1 pieces during slot-1 phase 1.
            wd_sched = {1: [0, 1]}
            for i in range(2, 6):
                wd_sched[i] = [i]
            wd_sched[6] = [6, 7]
            for i in range(7, 11):
                wd_sched[i] = [i + 1]

            for s in range(NSPLIT):
                C = Cs[s]
                # Phase 1: gateT/upT per FF-block, fused silu*up*combine_wt
                for fbl in range(FBH):
                    fb = s * FBH + fbl
                    wg_t = wg_tiles.pop(fb)
                    wu_t = wu_tiles.pop(fb)
                    if fb == 0:
                        # slot-1 token feed, now that slot-0 staging is done
                        nc.scalar.dma_start(xt[1][:], xt1_d[:])
                    # prefetch weights to depth 3: sync for fb 2..5
                    nfs = [2, 3] if fb == 0 else ([fb + 3] if fb + 3 < FBLK else [])
                    for nf in nfs:
                        eng = nc.sync if nf <= 5 else nc.scalar
                        nwg = wpool.tile([P, DSUB, P], F16, tag="wg", name="wg")
                        eng.dma_start(nwg[:], wg_d[nf])
                        nwu = wpool.tile([P, DSUB, P], F16, tag="wu", name="wu")
                        eng.dma_start(nwu[:], wu_d[nf])
                        wg_tiles[nf] = nwg
                        wu_tiles[nf] = nwu
                    for j in wd_sched.get(fb, []):
                        nc.scalar.dma_start(
                            wd_sb[j // FBH][:, j % FBH], wd_d[j // FBH][:, j % FBH]
                        )

                    for (toff, cs) in p1_chunks[s]:
                        src = xt[s]
                        pg = psum_pool.tile([P, 512], F32, tag="pg", name="pg")[:, :cs]
                        pu = psum_pool.tile([P, 512], F32, tag="pu", name="pu")[:, :cs]
                        for k in range(DSUB):
                            nc.tensor.matmul(
                                pg,
                                wg_t[:, k, :],
                                src[:, k, toff : toff + cs],
                                start=(k == 0),
                                stop=(k == DSUB - 1),
                            )
                        for k in range(DSUB):
                            nc.tensor.matmul(
                                pu,
                                wu_t[:, k, :],
                                src[:, k, toff : toff + cs],
                                start=(k == 0),
                                stop=(k == DSUB - 1),
                            )
                        if fb < 2:
                            # HAM insurance: keep the PE counted busy across
                            # supply-limited startup stutters so the clock
                            # gate doesn't re-throttle to 1.2GHz.
                            nc.tensor.matmul(warm_ps[:], warm_in[:, :P], warm_in[:])
                        sg = sg_pool.tile([P, 512], F32, tag="sg", name="sg")[:, :cs]
                        nc.scalar.activation(
                            sg, pg, mybir.ActivationFunctionType.Silu
                        )
                        tm = sg_pool.tile([P, 512], F32, tag="tm", name="tm")[:, :cs]
                        nc.vector.tensor_mul(tm, pu, wtb_sb[s][:, toff : toff + cs])
                        nc.vector.tensor_mul(
                            act[s][:, fbl, toff : toff + cs], sg, tm
                        )

                # Phase 2: y[d, t] = sum_ff Wd_half[d, ff] * act[ff, t];
                # output DMAs alternate between the two HWDGE engines so the
                # ~600ns dispatch serialization doesn't back up evictions.
                grp = 0
                for dblk in range(NDB):
                    for (c0, cs) in p2_chunks[s]:
                        py = psum_y_pool.tile([P, 512], F32, tag="py", name="py")[:, :cs]
                        for fs in range(FBH):
                            nc.tensor.matmul(
                                py,
                                wd_sb[s][:, fs, dblk, :],
                                act[s][:, fs, c0 : c0 + cs],
                                start=(fs == 0),
                                stop=(fs == FBH - 1),
                            )
                        y_sb = y_pool.tile([P, 512], F32, tag="ysb", name="ysb")[:, :cs]
                        nc.vector.tensor_copy(y_sb, py)
                        eng = nc.sync if grp % 2 == 0 else nc.scalar
                        eng.dma_start(y_d[s][dblk, :, c0 : c0 + cs], y_sb)
                        grp += 1

    nc.compile()
    return nc


def _shard_feed(h16, gp, up, dp, combine, routed, e, piece, C):
    """Build one (expert, FF-half) shard's DMA feeds, pre-laid-out to match the
    kernel's SBUF tile layouts exactly (every DMA contiguous)."""
    r = routed[e]
    n_e = len(r)
    idx_pad = np.zeros(C, np.int64)
    idx_pad[:n_e] = r
    wt_pad = np.zeros(C, np.float32)
    wt_pad[:n_e] = combine[e, r]
    hs = slice(piece * FH, (piece + 1) * FH)

    xg = h16[idx_pad]  # [C, D] fp16
    xt_feed = np.ascontiguousarray(xg.reshape(C, DSUB, P).transpose(2, 1, 0))
    wg_feed = np.ascontiguousarray(
        gp[e][hs, :].astype(np.float16).reshape(FBH, P, DSUB, P).transpose(0, 3, 2, 1)
    )
    wu_feed = np.ascontiguousarray(
        up[e][hs, :].astype(np.float16).reshape(FBH, P, DSUB, P).transpose(0, 3, 2, 1)
    )
    # wd_feed[p, fs, dblk, j] = down_proj[e][dblk*P+j, half*FH + fs*P + p]
    wd_feed = np.ascontiguousarray(
        dp[e][:, hs].astype(np.float16).reshape(NDB, P, FBH, P).transpose(3, 2, 0, 1)
    )
    wtb_feed = np.ascontiguousarray(
        np.broadcast_to(wt_pad.astype(np.float16), (P, C))
    )
    return xt_feed, wg_feed, wu_feed, wd_feed, wtb_feed


def kernel(hidden_states, top_k_index, top_k_weights, gate_proj, up_proj, down_proj):
    h = np.ascontiguousarray(np.asarray(hidden_states, dtype=np.float32))
    idx = np.asarray(top_k_index)
    wts = np.asarray(top_k_weights, dtype=np.float32)
    gp = np.asarray(gate_proj, dtype=np.float32)
    up = np.asarray(up_proj, dtype=np.float32)
    dp = np.asarray(down_proj, dtype=np.float32)
    assert h.shape == (T, D) and idx.shape == (T, TOPK)
    assert gp.shape == (E, FF, D) and dp.shape == (E, D, FF)

    # combine[e, t] = sum_k wts[t, k] * (idx[t, k] == e)
    combine = np.zeros((E, T), np.float32)
    for k in range(TOPK):
        np.add.at(combine, (idx[:, k], np.arange(T)), wts[:, k])

    routed = [np.nonzero(combine[e] > 0)[0] for e in range(E)]
    cnt = [len(r) for r in routed]

    # E*NSPLIT (expert, FF-half) shards, sorted by routed count; slot s takes
    # ranks [s*E, (s+1)*E).  Capacities are the exact per-tier max counts,
    # aligned to 8 tokens for 16B DMA rows (must also keep HEAD columns in
    # the slot-0 head tile and a nonzero tail chunk).
    shards = sorted(
        ((e, piece) for e in range(E) for piece in range(NSPLIT)),
        key=lambda sh: -cnt[sh[0]],
    )
    slots = [shards[s * E : (s + 1) * E] for s in range(NSPLIT)]
    pad8 = lambda n: -(-n // 8) * 8
    Cs = (
        max(136, pad8(max(cnt[e] for e, _ in slots[0]))),
        max(TAIL + 40, pad8(max(cnt[e] for e, _ in slots[1]))),
    )

    h16 = h.astype(np.float16)
    in_maps = []
    for core in range(E):
        m = {}
        for s in range(NSPLIT):
            e, piece = slots[s][core]
            xt_f, wg_f, wu_f, wd_f, wtb_f = _shard_feed(
                h16, gp, up, dp, combine, routed, e, piece, Cs[s]
            )
            if s == 0:
                m["xt0"] = xt_f
                wg_parts, wu_parts = [wg_f], [wu_f]
            else:
                m["xt1"] = xt_f
                wg_parts.append(wg_f)
                wu_parts.append(wu_f)
            m[f"wd{s}"] = wd_f
            m[f"wtb{s}"] = wtb_f
        m["wg"] = np.ascontiguousarray(np.concatenate(wg_parts, axis=0))
        m["wu"] = np.ascontiguousarray(np.concatenate(wu_parts, axis=0))
        in_maps.append(m)

    ys = _run_on_device(Cs, in_maps)

    out = np.zeros((T, D), np.float32)
    for core in range(E):
        for s in range(NSPLIT):
            e, piece = slots[s][core]
            r = routed[e]
            # y[s] is [NDB, P, C] fp32, D-major: D = dblk*P + p
            yv = ys[core][s].reshape(D, Cs[s])
            out[r] += yv[:, : len(r)].T
    return out


def _have_axon() -> bool:
    """The bass kernel executes via PJRT on the axon-tunneled NeuronCores.
    If the calling process pinned JAX_PLATFORMS=cpu (hiding them), fall back
    to a clean subprocess."""
    try:
        import jax

        return sum(1 for d in jax.devices() if getattr(d, "platform", "") != "cpu") >= E
    except Exception:
        return False


def _run_on_device(Cs: tuple, in_maps: list) -> list:
    global last_results
    if _have_axon():
        if Cs not in _program_cache:
            _program_cache[Cs] = _build_program(Cs)
        nc = _program_cache[Cs]
        last_results = run_bass_kernel_spmd(nc, in_maps, core_ids=list(range(E)))
        return [
            [last_results.results[c][f"y{s}"] for s in range(NSPLIT)]
            for c in range(E)
        ]

    import pickle
    import subprocess
    import tempfile

    d = tempfile.mkdtemp()
    inp, outp = os.path.join(d, "in.pkl"), os.path.join(d, "out.pkl")
    with open(inp, "wb") as f:
        pickle.dump((Cs, in_maps), f)
    env = dict(os.environ)
    env.pop("JAX_PLATFORMS", None)
    subprocess.run(
        [sys.executable, os.path.abspath(__file__), "--device-run", inp, outp],
        check=True,
        env=env,
    )
    with open(outp, "rb") as f:
        return pickle.load(f)


if __name__ == "__main__" and "--device-run" in sys.argv:
    import pickle

    _inp, _outp = sys.argv[2], sys.argv[3]
    with open(_inp, "rb") as f:
        _Cs, _in_maps = pickle.load(f)
    _nc = _build_program(_Cs)
    _res = run_bass_kernel_spmd(_nc, _in_maps, core_ids=list(range(E)))
    with open(_outp, "wb") as f:
        pickle.dump(
            [[_res.results[c][f"y{s}"] for s in range(NSPLIT)] for c in range(E)],
            f,
        )
